# revision 35
# baseline (speedup 1.0000x reference)
"""Trainium2 Bass kernel for nn_LogicLayer (ProductTNorm 'and' LogicLayer forward).

Math: y[b,o] = prod_i (1 - v[o,i]*u[b,i]),  v = sigmoid(w), u = 1 - atoms.
ln y[b,o] = sum_i ln(1 - v*u) ~= I*c0 + sum_{k=1..K} c_k * sum_i v^k[o,i] u^k[b,i]
so each polynomial term is a (B,I)x(I,O) matmul and the whole reduction runs on
TensorE instead of elementwise Ln on ScalarE (the 265us baseline approach).

Coefficients c_k: weighted least-squares fit of ln(1-x) on the input
distribution (weight ~ y^2 = the norm-relative metric), fitted against the
fp16-quantized basis the device actually computes (see fit_coeffs.py).

Per-core layout (8 cores, data-parallel over batch, B_loc=512):
  * inputs: a16T = fp16(atoms.T) slice (I, B_loc), lnvT = fp16(softplus(-w).T)
    (I, O); input DMA triggers split across the sync and scalar HWDGE queues.
  * moving side (DVE): plain fp16 tensor_mul chain m_k = m_{k-1} * base_k
    where base_k is m1n = a-1 or m1p = 1-a, chosen per step so that
    |m_k| = u^k and sign(m_k) = sign(c_k) exactly (no slow 1x-mode STT ops).
  * stationary side (ScalarE): sv_k = exp(-k*lnv + ln|c_k|) fp16 — one
    activation per term, all on the single Exp table set (the table load is
    pulled to t~0 by a dummy activation and overlaps the input DMAs).
  * TensorE: K*8 accumulating matmuls (2 i-tiles x 2 o-tiles x 2 batch
    halves, N=256) into 4 PSUM banks, fp32; garbage warm-up matmuls during
    the DMA window pull the PE HAM clock gate toward 8/8 for the real work.
  * tail: y = Exp(psum + I*c0) per quadrant; the bh=0 quadrants exp + DMA
    out while the bh=1 matmuls still run; all output triggers on the sync
    queue so they never block tail activations.
"""

from contextlib import ExitStack

import numpy as np

B, OUT, IN = 4096, 256, 256
NCORES = 8
B_LOC = B // NCORES  # 512 batch rows per core
K = 7
C0 = -0.00021011461496681297
CK = [
    -0.9936208153828796,
    -0.49080715992239754,
    -1.3323464496625737,
    7.192182074305612,
    -22.216926542381056,
    28.827825896133078,
    -14.511277540807844,
]
N_WARM_MM = 7

_COMPILED = {}


def _build_nc():
    import concourse.bacc as bacc
    import concourse.mybir as mybir
    import concourse.tile as tile

    AF = mybir.ActivationFunctionType
    F32 = mybir.dt.float32
    F16 = mybir.dt.float16
    MUL = mybir.AluOpType.mult

    sgn = [1.0 if c > 0 else -1.0 for c in CK]

    nc = bacc.Bacc(
        "TRN2", target_bir_lowering=False, debug=False, num_devices=NCORES
    )

    aT = nc.dram_tensor("aT", [IN, B_LOC], F16, kind="ExternalInput").ap()
    lnvT = nc.dram_tensor("lnvT", [IN, OUT], F16, kind="ExternalInput").ap()
    y = nc.dram_tensor("y", [OUT, B_LOC], F32, kind="ExternalOutput").ap()

    NIT = IN // 128  # 2 i-tiles
    NOT_ = OUT // 128  # 2 o-tiles

    with tile.TileContext(nc) as tc, ExitStack() as es:
        const = es.enter_context(tc.tile_pool(name="const", bufs=1))
        mk_pool = es.enter_context(tc.tile_pool(name="mk", bufs=3))
        sv_pool = es.enter_context(tc.tile_pool(name="sv", bufs=K))
        ps_pool = es.enter_context(tc.tile_pool(name="ps", bufs=1, space="PSUM"))

        # input DMAs split across the two HWDGE rings: scalar ring carries
        # lnv it0 (triggered before the table-load dummy so the transfer
        # overlaps the load) then atoms it1; sync ring carries atoms it0 then
        # lnv it1.  ~1us trigger->packet lag, ~200GB/s striped transfers.
        lnv = const.tile([128, NIT * OUT], F16, name="lnv", tag="lnv")
        a16 = const.tile([128, NIT * B_LOC], F16, name="a16", tag="a16")
        nc.scalar.dma_start(lnv[:, 0:OUT], lnvT[0:128, :])

        # scalar queue: force the (single) Exp table load while DMAs run
        scratch = const.tile([128, 1], F32, name="scratch", tag="scratch")
        zero_ap = nc.const_aps.tensor(0.0, (128, 1))
        nc.scalar.activation(scratch[:], zero_ap, AF.Exp)

        nc.scalar.dma_start(a16[:, B_LOC : 2 * B_LOC], aT[128:256, :])
        nc.sync.dma_start(a16[:, 0:B_LOC], aT[0:128, :])
        nc.sync.dma_start(lnv[:, OUT : 2 * OUT], lnvT[128:256, :])

        # gpsimd: bias constants for the stationary activations + warm tile
        warm = const.tile([128, 512], F16, name="warm", tag="warm")
        nc.gpsimd.memset(warm[:], 0.0)
        lnck = const.tile([128, K], F32, name="lnck", tag="lnck")
        for k in range(K):
            nc.gpsimd.memset(lnck[:, k : k + 1], float(np.log(abs(CK[k]))))
        bias_c0 = const.tile([128, 1], F32, name="bias_c0", tag="bias_c0")
        nc.gpsimd.memset(bias_c0[:], float(IN * C0))

        # warm-up garbage matmuls lift the PE HAM clock gate during DMA wait
        warm_ps = ps_pool.tile([128, 512], F32, name="warm_ps", tag="warm_ps")
        for _ in range(N_WARM_MM):
            nc.tensor.matmul(
                warm_ps[:], lhsT=warm[:, 0:128], rhs=warm[:], start=True, stop=True
            )

        # stationaries: sv_k = fp16(exp(-k*lnv + ln|c_k|)), always positive;
        # the sign of c_k rides on the moving chain (see below)
        svs = []
        for k in range(1, K + 1):
            sv = sv_pool.tile([128, NIT * OUT], F16, name="sv", tag="sv")
            if k == 1:  # split halves so the first matmul starts earlier
                for it in range(NIT):
                    sl = slice(it * OUT, (it + 1) * OUT)
                    nc.scalar.activation(
                        sv[:, sl], lnv[:, sl], AF.Exp, scale=-1.0,
                        bias=lnck[:, 0:1],
                    )
            else:
                nc.scalar.activation(
                    sv[:], lnv[:], AF.Exp, scale=-float(k), bias=lnck[:, k - 1 : k]
                )
            svs.append(sv)

        # moving side: m_k = sign(c_k) * u^k via a plain-TT chain multiplying
        # by one of two base tiles: m1n = -(u) = a-1 (flips sign) or
        # m1p = +u = 1-a (keeps sign); the step-k base is chosen so that
        # sign(m_k) = sign(c_k) exactly.
        chi = [sgn[0]] + [sgn[k - 1] * sgn[k - 2] for k in range(2, K + 1)]
        need_n = any(c < 0 for c in chi)
        need_p = any(c > 0 for c in chi)
        m1n = const.tile([128, NIT * B_LOC], F16, name="m1n", tag="m1n")
        m1p = const.tile([128, NIT * B_LOC], F16, name="m1p", tag="m1p")
        primary, secondary = (m1n, m1p) if sgn[0] < 0 else (m1p, m1n)
        psc = (1.0, -1.0) if sgn[0] < 0 else (-1.0, 1.0)
        for it in range(NIT):
            sl = slice(it * B_LOC, (it + 1) * B_LOC)
            nc.vector.tensor_scalar(
                primary[:, sl], a16[:, sl], psc[0], psc[1], MUL, mybir.AluOpType.add
            )
        if need_n and need_p:
            for it in range(NIT):
                sl = slice(it * B_LOC, (it + 1) * B_LOC)
                nc.vector.tensor_scalar_mul(secondary[:, sl], primary[:, sl], -1.0)

        # one PSUM bank per (o-tile, batch-half) quadrant: the bh=0 banks
        # close during the last term, so half the output exps + DMAs overlap
        # the remaining matmuls.  Tiles are bank-sized (512 f32) with only
        # the first 256 columns used, to keep PE writes and ScalarE reads on
        # different physical banks.
        BH = B_LOC // 2  # 256
        psums = {}
        for ot in range(NOT_):
            for bh in range(2):
                psums[(ot, bh)] = ps_pool.tile(
                    [128, 512], F32, name=f"ps{ot}{bh}", tag=f"ps{ot}{bh}"
                )

        mk_prev = primary
        for k in range(1, K + 1):
            if k == 1:
                mk = primary
            else:
                base = m1n if chi[k - 1] < 0 else m1p
                mk = mk_pool.tile([128, NIT * B_LOC], F16, name="mk", tag="mk")
                nc.vector.tensor_mul(mk[:], mk_prev[:], base[:])
            mk_prev = mk
            sv = svs[k - 1]
            if k < K:
                order = [(it, ot, bh) for it in range(NIT) for ot in range(NOT_)
                         for bh in range(2)]
            else:  # last term: close the bh=0 banks first
                order = [(it, ot, bh) for bh in range(2) for it in range(NIT)
                         for ot in range(NOT_)]
            for it, ot, bh in order:
                nc.tensor.matmul(
                    psums[(ot, bh)][:, 0:BH],
                    lhsT=sv[:, it * OUT + ot * 128 : it * OUT + ot * 128 + 128],
                    rhs=mk[:, it * B_LOC + bh * BH : it * B_LOC + bh * BH + BH],
                    start=(k == 1 and it == 0),
                    stop=(k == K and it == NIT - 1),
                )

        # tail: y = exp(psum + I*c0) per quadrant; bh=0 quadrants flow out
        # while the bh=1 matmuls still run.  Quadrants alternate rings.
        y_sb = const.tile([128, NOT_ * B_LOC], F32, name="y_sb", tag="y_sb")
        for bh in range(2):
            for ot in range(NOT_):
                sl = slice(ot * B_LOC + bh * BH, ot * B_LOC + bh * BH + BH)
                nc.scalar.activation(
                    y_sb[:, sl], psums[(ot, bh)][:, 0:BH], AF.Exp,
                    bias=bias_c0[:, 0:1],
                )
                # all output triggers on the sync queue: one ring sustains
                # ~200GB/s, and triggers on the scalar queue would block the
                # remaining tail exps behind them
                nc.sync.dma_start(
                    y[ot * 128 : (ot + 1) * 128, bh * BH : (bh + 1) * BH],
                    y_sb[:, sl],
                )

    nc.compile()
    return nc


def get_nc():
    if "nc" not in _COMPILED:
        _COMPILED["nc"] = _build_nc()
    return _COMPILED["nc"]


def make_in_maps(atoms: np.ndarray, weights: np.ndarray):
    atoms = np.asarray(atoms)
    w32 = np.asarray(weights).astype(np.float32, copy=False)
    aT = np.ascontiguousarray(atoms.T.astype(np.float16))
    lnvT = np.ascontiguousarray(np.log1p(np.exp(-w32)).T.astype(np.float16))
    in_maps = []
    for c in range(NCORES):
        aT_sl = np.ascontiguousarray(aT[:, c * B_LOC : (c + 1) * B_LOC])
        in_maps.append({"aT": aT_sl, "lnvT": lnvT})
    return in_maps


def run(atoms: np.ndarray, weights: np.ndarray, **spmd_kwargs):
    from concourse.bass_utils import run_bass_kernel_spmd

    nc = get_nc()
    in_maps = make_in_maps(atoms, weights)
    res = run_bass_kernel_spmd(nc, in_maps, core_ids=list(range(NCORES)), **spmd_kwargs)
    out = np.empty((B, OUT), np.float32)
    for c in range(NCORES):
        out[c * B_LOC : (c + 1) * B_LOC, :] = res.results[c]["y"].T
    return out, res


def kernel(atoms: np.ndarray, weights: np.ndarray) -> np.ndarray:
    out, _ = run(atoms, weights)
    return out


# revision 36
# speedup vs baseline: 1.1810x; 1.1810x over previous
"""Trainium2 Bass kernel for nn_LogicLayer (ProductTNorm 'and' LogicLayer forward).

Math: y[b,o] = prod_i (1 - v[o,i]*u[b,i]),  v = sigmoid(w), u = 1 - atoms.
ln y[b,o] = sum_i ln(1 - v*u) ~= I*c0 + sum_{k=1..K} c_k * sum_i v^k[o,i] u^k[b,i]
so each polynomial term is a (B,I)x(I,O) matmul and the whole reduction runs on
TensorE instead of elementwise Ln on ScalarE (the 265us baseline approach).

Coefficients c_k: weighted least-squares fit of ln(1-x) on the input
distribution (weight ~ y^2 = the norm-relative metric), fitted against the
fp16-quantized basis the device actually computes (see fit_coeffs.py).

Per-core layout (8 cores, data-parallel over batch, B_loc=512):
  * inputs: a16T = fp16(atoms.T) slice (I, B_loc), lnvT = fp16(softplus(-w).T)
    (I, O); input DMA triggers split across the sync and scalar HWDGE queues.
  * moving side (DVE): plain fp16 tensor_mul chain m_k = m_{k-1} * base_k
    where base_k is m1n = a-1 or m1p = 1-a, chosen per step so that
    |m_k| = u^k and sign(m_k) = sign(c_k) exactly (no slow 1x-mode STT ops,
    no GpSimd elementwise — it contends with DVE for SBUF ports).
  * stationary side (ScalarE): sv_k = exp(-k*lnv + ln|c_k|) fp16 — one
    activation per term, all on the single Exp table set (the table load is
    pulled to t~0 by a dummy activation and overlaps the input DMAs).
  * TensorE: K*8 accumulating matmuls (2 i-tiles x 2 o-tiles x 2 batch
    halves, N=256) into 4 PSUM banks, fp32; garbage warm-up matmuls during
    the DMA window pull the PE HAM clock gate toward 8/8 for the real work.
  * tail: y = Exp(psum + I*c0) per quadrant; the bh=0 quadrants exp + DMA
    out while the bh=1 matmuls still run; all output triggers on the sync
    queue so they never block tail activations.
"""

from contextlib import ExitStack

import numpy as np

B, OUT, IN = 4096, 256, 256
NCORES = 8
B_LOC = B // NCORES  # 512 batch rows per core
K = 7
C0 = -0.00021011461496681297
CK = [
    -0.9936208153828796,
    -0.49080715992239754,
    -1.3323464496625737,
    7.192182074305612,
    -22.216926542381056,
    28.827825896133078,
    -14.511277540807844,
]
N_WARM_MM = 7

_COMPILED = {}


def _build_nc():
    import concourse.bacc as bacc
    import concourse.mybir as mybir
    import concourse.tile as tile

    AF = mybir.ActivationFunctionType
    F32 = mybir.dt.float32
    F16 = mybir.dt.float16
    MUL = mybir.AluOpType.mult

    sgn = [1.0 if c > 0 else -1.0 for c in CK]

    nc = bacc.Bacc(
        "TRN2", target_bir_lowering=False, debug=False, num_devices=NCORES
    )

    aT = nc.dram_tensor("aT", [IN, B_LOC], F16, kind="ExternalInput").ap()
    lnvT = nc.dram_tensor("lnvT", [IN, OUT], F16, kind="ExternalInput").ap()
    y = nc.dram_tensor("y", [OUT, B_LOC], F32, kind="ExternalOutput").ap()

    NIT = IN // 128  # 2 i-tiles
    NOT_ = OUT // 128  # 2 o-tiles

    with tile.TileContext(nc) as tc, ExitStack() as es:
        const = es.enter_context(tc.tile_pool(name="const", bufs=1))
        mk_pool = es.enter_context(tc.tile_pool(name="mk", bufs=3))
        sv_pool = es.enter_context(tc.tile_pool(name="sv", bufs=K))
        ps_pool = es.enter_context(tc.tile_pool(name="ps", bufs=1, space="PSUM"))

        # input DMAs split across the two HWDGE rings: scalar ring carries
        # lnv it0 (triggered before the table-load dummy so the transfer
        # overlaps the load) then atoms it1; sync ring carries atoms it0 then
        # lnv it1.  ~1us trigger->packet lag, ~200GB/s striped transfers.
        lnv = const.tile([128, NIT * OUT], F16, name="lnv", tag="lnv")
        a16 = const.tile([128, NIT * B_LOC], F16, name="a16", tag="a16")
        nc.scalar.dma_start(lnv[:, 0:OUT], lnvT[0:128, :])

        # scalar queue: force the (single) Exp table load while DMAs run
        scratch = const.tile([128, 1], F32, name="scratch", tag="scratch")
        zero_ap = nc.const_aps.tensor(0.0, (128, 1))
        nc.scalar.activation(scratch[:], zero_ap, AF.Exp)

        nc.scalar.dma_start(a16[:, B_LOC : 2 * B_LOC], aT[128:256, :])
        nc.sync.dma_start(a16[:, 0:B_LOC], aT[0:128, :])
        nc.sync.dma_start(lnv[:, OUT : 2 * OUT], lnvT[128:256, :])

        # gpsimd: bias constants for the stationary activations + warm tile
        warm = const.tile([128, 512], F16, name="warm", tag="warm")
        nc.gpsimd.memset(warm[:], 0.0)
        lnck = const.tile([128, K], F32, name="lnck", tag="lnck")
        for k in range(K):
            nc.gpsimd.memset(lnck[:, k : k + 1], float(np.log(abs(CK[k]))))
        bias_c0 = const.tile([128, 1], F32, name="bias_c0", tag="bias_c0")
        nc.gpsimd.memset(bias_c0[:], float(IN * C0))

        # warm-up garbage matmuls lift the PE HAM clock gate during DMA wait
        warm_ps = ps_pool.tile([128, 512], F32, name="warm_ps", tag="warm_ps")
        for _ in range(N_WARM_MM):
            nc.tensor.matmul(
                warm_ps[:], lhsT=warm[:, 0:128], rhs=warm[:], start=True, stop=True
            )

        # stationaries: sv_k = fp16(exp(-k*lnv + ln|c_k|)), always positive;
        # the sign of c_k rides on the moving chain (see below)
        svs = []
        for k in range(1, K + 1):
            sv = sv_pool.tile([128, NIT * OUT], F16, name="sv", tag="sv")
            if k == 1:  # split halves so the first matmul starts earlier
                for it in range(NIT):
                    sl = slice(it * OUT, (it + 1) * OUT)
                    nc.scalar.activation(
                        sv[:, sl], lnv[:, sl], AF.Exp, scale=-1.0,
                        bias=lnck[:, 0:1],
                    )
            else:
                nc.scalar.activation(
                    sv[:], lnv[:], AF.Exp, scale=-float(k), bias=lnck[:, k - 1 : k]
                )
            svs.append(sv)

        # moving side: m_k = sign(c_k) * u^k via a plain-TT chain multiplying
        # by one of two base tiles: m1n = -(u) = a-1 (flips sign) or
        # m1p = +u = 1-a (keeps sign); the step-k base is chosen so that
        # sign(m_k) = sign(c_k) exactly.
        chi = [sgn[0]] + [sgn[k - 1] * sgn[k - 2] for k in range(2, K + 1)]
        need_n = any(c < 0 for c in chi)
        need_p = any(c > 0 for c in chi)
        m1n = const.tile([128, NIT * B_LOC], F16, name="m1n", tag="m1n")
        m1p = const.tile([128, NIT * B_LOC], F16, name="m1p", tag="m1p")
        primary, secondary = (m1n, m1p) if sgn[0] < 0 else (m1p, m1n)
        psc = (1.0, -1.0) if sgn[0] < 0 else (-1.0, 1.0)
        for it in range(NIT):
            sl = slice(it * B_LOC, (it + 1) * B_LOC)
            nc.vector.tensor_scalar(
                primary[:, sl], a16[:, sl], psc[0], psc[1], MUL, mybir.AluOpType.add
            )
        if need_n and need_p:
            for it in range(NIT):
                sl = slice(it * B_LOC, (it + 1) * B_LOC)
                nc.vector.tensor_scalar_mul(secondary[:, sl], primary[:, sl], -1.0)

        # one PSUM bank per (o-tile, batch-half) quadrant: the bh=0 banks
        # close during the last term, so half the output exps + DMAs overlap
        # the remaining matmuls.  Tiles are bank-sized (512 f32) with only
        # the first 256 columns used, to keep PE writes and ScalarE reads on
        # different physical banks.
        BH = B_LOC // 2  # 256
        psums = {}
        for ot in range(NOT_):
            for bh in range(2):
                psums[(ot, bh)] = ps_pool.tile(
                    [128, 512], F32, name=f"ps{ot}{bh}", tag=f"ps{ot}{bh}"
                )

        mk_prev = primary
        for k in range(1, K + 1):
            if k == 1:
                mk = primary
            else:
                base = m1n if chi[k - 1] < 0 else m1p
                mk = mk_pool.tile([128, NIT * B_LOC], F16, name="mk", tag="mk")
                nc.vector.tensor_mul(mk[:], mk_prev[:], base[:])
            mk_prev = mk
            sv = svs[k - 1]
            if k < K:
                order = [(it, ot, bh) for it in range(NIT) for ot in range(NOT_)
                         for bh in range(2)]
            else:  # last term: close the bh=0 banks first
                order = [(it, ot, bh) for bh in range(2) for it in range(NIT)
                         for ot in range(NOT_)]
            for it, ot, bh in order:
                nc.tensor.matmul(
                    psums[(ot, bh)][:, 0:BH],
                    lhsT=sv[:, it * OUT + ot * 128 : it * OUT + ot * 128 + 128],
                    rhs=mk[:, it * B_LOC + bh * BH : it * B_LOC + bh * BH + BH],
                    start=(k == 1 and it == 0),
                    stop=(k == K and it == NIT - 1),
                )

        # tail: y = exp(psum + I*c0) per quadrant; bh=0 quadrants flow out
        # while the bh=1 matmuls still run.  Quadrants alternate rings.
        y_sb = const.tile([128, NOT_ * B_LOC], F32, name="y_sb", tag="y_sb")
        for bh in range(2):
            for ot in range(NOT_):
                sl = slice(ot * B_LOC + bh * BH, ot * B_LOC + bh * BH + BH)
                nc.scalar.activation(
                    y_sb[:, sl], psums[(ot, bh)][:, 0:BH], AF.Exp,
                    bias=bias_c0[:, 0:1],
                )
                # all output triggers on the sync queue: one ring sustains
                # ~200GB/s, and triggers on the scalar queue would block the
                # remaining tail exps behind them
                nc.sync.dma_start(
                    y[ot * 128 : (ot + 1) * 128, bh * BH : (bh + 1) * BH],
                    y_sb[:, sl],
                )

    nc.compile()
    return nc


def get_nc():
    if "nc" not in _COMPILED:
        _COMPILED["nc"] = _build_nc()
    return _COMPILED["nc"]


def make_in_maps(atoms: np.ndarray, weights: np.ndarray):
    atoms = np.asarray(atoms)
    w32 = np.asarray(weights).astype(np.float32, copy=False)
    aT = np.ascontiguousarray(atoms.T.astype(np.float16))
    lnvT = np.ascontiguousarray(np.log1p(np.exp(-w32)).T.astype(np.float16))
    in_maps = []
    for c in range(NCORES):
        aT_sl = np.ascontiguousarray(aT[:, c * B_LOC : (c + 1) * B_LOC])
        in_maps.append({"aT": aT_sl, "lnvT": lnvT})
    return in_maps


def run(atoms: np.ndarray, weights: np.ndarray, **spmd_kwargs):
    from concourse.bass_utils import run_bass_kernel_spmd

    nc = get_nc()
    in_maps = make_in_maps(atoms, weights)
    res = run_bass_kernel_spmd(nc, in_maps, core_ids=list(range(NCORES)), **spmd_kwargs)
    out = np.empty((B, OUT), np.float32)
    for c in range(NCORES):
        out[c * B_LOC : (c + 1) * B_LOC, :] = res.results[c]["y"].T
    return out, res


def kernel(atoms: np.ndarray, weights: np.ndarray) -> np.ndarray:
    out, _ = run(atoms, weights)
    return out
